# revision 6
# baseline (speedup 1.0000x reference)
"""Trainium2 Bass kernel for nn_Attention_386547057357 (Transformer-XL style
relative-position attention).

Sharding: data-parallel over batch - core c computes batch element c.
All weights replicated per core.

Math (per batch element):
    X = [memory; x]  (1024, 512)
    qT[hd, n] (x tokens only);  kT[hd, m];  V[m, hd]   (via W_qkv)
    qhatT = qT + u_emb;  qtldT = qT + v_emb   (per-hd ACT bias pass)
    RW = R @ W_rel  (1024, 512)   [host-computed]
    bd[n, r'] = qtld[n] . RW[1023 - r']       (= term_b + term_d)
    ac[n, m]  = qhat[n] . k[m]                (= term_a + term_c)
    attn[n, m] = exp(s*ac[n, m]) * exp(s*bd)[n, 255 - n + m]
      - the shift is a bf16 DRAM round trip: write exp(s*bd) rows with
        stride 1281 (cols 1024:1281 zero), read [[1280,128],[1,1024]] at
        offset 255.  Zero pad = causal mask for free.
      - merge is ONE DVE scalar_tensor_tensor pass (all bf16, 2x mode)
        whose accum_out also emits the softmax denominator per row.
    normalize: attn *= recip(rowsum)  (per-partition tensor_scalar)
    attnT = transpose(attn)  (gpsimd dma_gather)
    out[n, :] = sum_h (attnT_h^T V_h) @ W_out_h + b_out

DMA layout: one [2048, 1281] bf16 scratch for all 8 heads; 4 pad writes,
1 write + 1 shifted read per head (3-dim matched APs).  Issue spread:
wqkv/reads/out on Sync, xt on Vector, small consts on Scalar,
rwt/pads/writes/gathers on GpSimd.
"""

import sys

sys.path.insert(0, "/opt/trn_rl_repo")

import numpy as np
import ml_dtypes

import concourse.bass as bass
import concourse.mybir as mybir
import concourse.tile as tile
from concourse import bacc, library_config
from concourse.bass_utils import run_bass_kernel_spmd
from concourse.tile_rust import add_dep_helper

BF16 = ml_dtypes.bfloat16
F32 = np.float32

DIM = 512
NHEAD = 8
DHEAD = 64
CTX = 1024
NOCT = 11
B = 8
SEQ = 256
MEM = 768
TOT = MEM + SEQ  # 1024
SCALE = DHEAD ** -0.5  # 0.125
RSTRIDE = 1281  # bd scratch row stride (1024 data + 257 zero pad)

dt = mybir.dt
AF = mybir.ActivationFunctionType
ALU = mybir.AluOpType


# ---------------------------------------------------------------- host consts
def _positional_encoding():
    coords = np.arange(CTX, dtype=F32)[:, None]
    octaves = np.arange(1 - NOCT, 1, dtype=F32)
    mult = ((2.0 ** octaves) * np.pi).astype(F32)
    scaled = (coords * mult[None, :]).astype(F32)
    return np.concatenate([np.sin(scaled), np.cos(scaled)], axis=-1).astype(F32)


def _chunked(w, nchunk):
    """(128*nchunk, F) -> (128, nchunk, F) with [p, c, f] = w[128c + p, f]."""
    f = w.shape[1]
    return np.ascontiguousarray(w.reshape(nchunk, 128, f).transpose(1, 0, 2))


# ---------------------------------------------------------------- bass program
def build_program():
    nc = bacc.Bacc("TRN2", target_bir_lowering=False, debug=False)

    xt_d = nc.dram_tensor("xt", [128, 4, TOT], dt.bfloat16, kind="ExternalInput")
    wqkv_d = nc.dram_tensor("wqkv", [128, 4, 1536], dt.bfloat16, kind="ExternalInput")
    rwt_d = nc.dram_tensor("rwt", [128, 4, CTX], dt.bfloat16, kind="ExternalInput")
    wout_d = nc.dram_tensor("wout", [128, 4, 512], dt.bfloat16, kind="ExternalInput")
    bout_d = nc.dram_tensor("bout", [128, 512], dt.float32, kind="ExternalInput")
    u2_d = nc.dram_tensor("u2", [128, 1], dt.float32, kind="ExternalInput")
    v2_d = nc.dram_tensor("v2", [128, 1], dt.float32, kind="ExternalInput")
    gidx_d = nc.dram_tensor("gidx", [128, 16], dt.int16, kind="ExternalInput")
    out_d = nc.dram_tensor("out", [SEQ, 512], dt.float32, kind="ExternalOutput")

    with tile.TileContext(nc) as tc:
        _body(tc, xt_d, wqkv_d, rwt_d, wout_d, bout_d, u2_d, v2_d, gidx_d,
              out_d)
    nc.compile()
    return nc


def _body(tc, xt_d, wqkv_d, rwt_d, wout_d, bout_d, u2_d, v2_d, gidx_d, out_d):
    nc = tc.nc
    from contextlib import ExitStack

    with ExitStack() as ctx:
        consts = ctx.enter_context(tc.tile_pool(name="consts", bufs=1))

        # ---- constants / weights, issue spread across engines
        wqkv = consts.tile([128, 4, 1536], dt.bfloat16)
        xt = consts.tile([128, 4, TOT], dt.bfloat16)
        rwt = consts.tile([128, 4, CTX], dt.bfloat16)
        u2 = consts.tile([128, 1], dt.float32)
        v2 = consts.tile([128, 1], dt.float32)
        gidx = consts.tile([128, 16], dt.int16)
        wout = consts.tile([128, 4, 512], dt.bfloat16)
        bout = consts.tile([128, 512], dt.float32)

        nc.sync.dma_start(wqkv[:, 0:2, :], wqkv_d.ap()[:, 0:2, :])
        nc.scalar.dma_start(xt[:, 0:2, :], xt_d.ap()[:, 0:2, :])
        nc.gpsimd.dma_start(rwt[:], rwt_d.ap())
        nc.sync.dma_start(wqkv[:, 2:4, :], wqkv_d.ap()[:, 2:4, :])
        nc.scalar.dma_start(xt[:, 2:4, :], xt_d.ap()[:, 2:4, :])
        nc.scalar.dma_start(u2[:], u2_d.ap())
        nc.scalar.dma_start(v2[:], v2_d.ap())
        nc.scalar.dma_start(gidx[:], gidx_d.ap())
        nc.scalar.dma_start(wout[:], wout_d.ap())
        nc.scalar.dma_start(bout[:], bout_d.ap())

        # persistent intermediates
        qhatT = consts.tile([128, 4, SEQ], dt.bfloat16)  # (q+u)^T  [hd, n]
        qtldT = consts.tile([128, 4, SEQ], dt.bfloat16)  # (q+v)^T  [hd, n]
        kT = consts.tile([128, 4, TOT], dt.bfloat16)     # k^T      [hd, m]
        vv = consts.tile([128, 8, 512], dt.bfloat16)     # V        [m, hd]
        zpad = consts.tile([128, 1028], dt.bfloat16)     # 4x257 pad rows
        avt = consts.tile([128, 4, SEQ], dt.bfloat16)    # attnV^T  [hd, n]

        nc.vector.memset(zpad[:], 0.0)

        # gpsimd library for dma_gather
        lib_inst = nc.gpsimd.load_library(library_config.mlp)

        with (
            tc.tile_pool(name="mps", bufs=3, space="PSUM") as mps,
            tc.tile_pool(name="bdt", bufs=3) as bdtp,
            tc.tile_pool(name="ebd", bufs=6) as ebdp,
            tc.tile_pool(name="eac", bufs=4) as eacp,
            tc.tile_pool(name="atn", bufs=2) as atnp,
            tc.tile_pool(name="at2", bufs=2) as at2p,
            tc.tile_pool(name="att", bufs=3) as attp,
            tc.tile_pool(name="sm", bufs=3) as smp,
            tc.tile_pool(name="bdd", bufs=1, space="DRAM") as bddp,
        ):
            # one bd scratch for all heads; head h owns rows 256h..256h+255
            bdd = bddp.tile([8 * SEQ, RSTRIDE], dt.bfloat16, tag="bdd")
            for quad in range(4):
                nc.gpsimd.dma_start(
                    bass.AP(bdd.tensor,
                            bdd.offset + 512 * quad * RSTRIDE + TOT,
                            [[4 * RSTRIDE, 128], [RSTRIDE, 4], [1, 257]]),
                    bass.AP(zpad.tensor, zpad.offset,
                            [[1028, 128], [257, 4], [1, 257]]))

            # ---------------- q projection: qT[hd, n] packed 4 hp per tile
            qps = mps.tile([128, 1024], dt.float32, tag="m")
            for hp in range(4):
                for ch in range(4):
                    nc.tensor.matmul(qps[:, 256 * hp:256 * (hp + 1)],
                                     wqkv[:, ch, 128 * hp:128 * (hp + 1)],
                                     xt[:, ch, MEM:TOT],
                                     start=(ch == 0), stop=(ch == 3))
            for hp in range(4):
                src = qps[:, 256 * hp:256 * (hp + 1)]
                nc.scalar.activation(qtldT[:, hp, :], src, func=AF.Identity,
                                     bias=v2[:])
            for hp in range(4):
                src = qps[:, 256 * hp:256 * (hp + 1)]
                nc.scalar.activation(qhatT[:, hp, :], src, func=AF.Identity,
                                     bias=u2[:])

            def emit_k(hp):
                ps = mps.tile([128, 1024], dt.float32, tag="m")
                for mh in range(2):
                    for ch in range(4):
                        nc.tensor.matmul(
                            ps[:, 512 * mh:512 * (mh + 1)],
                            wqkv[:, ch, 512 + 128 * hp:512 + 128 * (hp + 1)],
                            xt[:, ch, 512 * mh:512 * (mh + 1)],
                            start=(ch == 0), stop=(ch == 3))
                nc.vector.tensor_copy(kT[:, hp, :], ps[:])

            def emit_v(mc0):
                ps = mps.tile([128, 1024], dt.float32, tag="m")
                for k2 in range(2):
                    for ch in range(4):
                        nc.tensor.matmul(
                            ps[:, 512 * k2:512 * (k2 + 1)],
                            xt[:, ch, 128 * (mc0 + k2):128 * (mc0 + k2 + 1)],
                            wqkv[:, ch, 1024:1536],
                            start=(ch == 0), stop=(ch == 3))
                nc.vector.tensor_copy(vv[:, mc0:mc0 + 2, :], ps[:])

            def emit_bd_mm(h):
                """bd matmuls -> exp(bf16) into a [128, 2, 1024] staging."""
                hp, pb = h // 2, 64 * (h % 2)
                bdt = bdtp.tile([128, 2, TOT], dt.bfloat16, tag="bdt")
                bdts[h] = bdt
                for n2 in range(2):
                    ps = mps.tile([128, 1024], dt.float32, tag="m")
                    for rh in range(2):
                        nc.tensor.matmul(
                            ps[:, 512 * rh:512 * (rh + 1)],
                            qtldT[pb:pb + 64, hp, 128 * n2:128 * (n2 + 1)],
                            rwt[pb:pb + 64, hp, 512 * rh:512 * (rh + 1)],
                            start=True, stop=True)
                    nc.scalar.activation(bdt[:, n2, :], ps[:], func=AF.Exp,
                                         scale=SCALE)

            def emit_bd_write(h):
                nc.gpsimd.dma_start(
                    bass.AP(bdd.tensor, bdd.offset + 256 * h * RSTRIDE,
                            [[RSTRIDE, 128], [128 * RSTRIDE, 2], [1, TOT]]),
                    bdts[h][:])

            def emit_read(h):
                """shifted readback: flat = 255 + 1280 n + m (per head)."""
                t = ebdp.tile([128, 2, TOT], dt.bfloat16, tag="ebd")
                nc.sync.dma_start(
                    t[:],
                    bass.AP(bdd.tensor,
                            bdd.offset + 256 * h * RSTRIDE + 255,
                            [[RSTRIDE - 1, 128], [128 * (RSTRIDE - 1), 2],
                             [1, TOT]]))
                ebds[h] = t

            def emit_ac(h):
                hp, pb = h // 2, 64 * (h % 2)
                eac = eacp.tile([128, 2, TOT], dt.bfloat16, tag="eac")
                eacs[h] = eac
                for n2 in range(2):
                    ps = mps.tile([128, 1024], dt.float32, tag="m")
                    for mh in range(2):
                        nc.tensor.matmul(
                            ps[:, 512 * mh:512 * (mh + 1)],
                            qhatT[pb:pb + 64, hp, 128 * n2:128 * (n2 + 1)],
                            kT[pb:pb + 64, hp, 512 * mh:512 * (mh + 1)],
                            start=True, stop=True)
                    nc.scalar.activation(eac[:, n2, :], ps[:], func=AF.Exp,
                                         scale=SCALE)

            def emit_merge(h):
                """merge + rowsum (one DVE pass), recip, normalize, gather."""
                attn = atnp.tile([128, 2, TOT], dt.bfloat16, tag="attn")
                rs = smp.tile([128, 2], dt.float32, tag="rs")
                for n2 in range(2):
                    nc.vector.scalar_tensor_tensor(
                        attn[:, n2, :], eacs[h][:, n2, :], 1.0,
                        ebds[h][:, n2, :], op0=ALU.mult, op1=ALU.mult,
                        accum_out=rs[:, n2:n2 + 1])
                rrec = smp.tile([128, 2], dt.float32, tag="rrec")
                nc.vector.reciprocal(rrec[:], rs[:])
                attn2 = at2p.tile([128, 2, TOT], dt.bfloat16, tag="attn2")
                for n2 in range(2):
                    nc.vector.tensor_scalar_mul(attn2[:, n2, :],
                                                attn[:, n2, :],
                                                rrec[:, n2:n2 + 1])
                attns[h] = attn2

            def emit_gather(h):
                attnT = attp.tile([128, 8, SEQ], dt.bfloat16, tag="attnT")
                g = nc.gpsimd.dma_gather(
                    out_ap=attnT[:], in_ap=attns[h][:], idxs_ap=gidx[:],
                    num_idxs=SEQ, num_idxs_reg=SEQ, elem_size=TOT,
                    transpose=True, sbuf_tokens_per_rank=128,
                    sbuf_free_dim_per_rank=2 * TOT,
                    sbuf_free_dim_pad_per_rank=0, sbuf_byte_offset=0)
                add_dep_helper(g.ins, lib_inst.ins,
                               reason="dma_gather needs mlp gpsimd library")
                attnTs[h] = attnT

            def emit_pv(hp):
                """PV for head pair hp -> avt[:, hp, :] (bf16)."""
                pvt = mps.tile([128, 1024], dt.float32, tag="m")
                for par in range(2):
                    h = 2 * hp + par
                    pb = 64 * par
                    for mc in range(8):
                        nc.tensor.matmul(
                            pvt[pb:pb + 64, 0:SEQ],
                            vv[:, mc, 64 * h:64 * (h + 1)],
                            attnTs[h][:, mc, :],
                            start=(mc == 0), stop=(mc == 7),
                            tile_position=(0, pb))
                nc.vector.tensor_copy(avt[:, hp, :], pvt[:, 0:SEQ])

            bdts, ebds, eacs, attns, attnTs = {}, {}, {}, {}, {}

            # ---------------- schedule (PE stream stays dense; bd round
            # trips and gathers overlap projection matmuls)
            emit_k(0)
            emit_k(1)
            for h in range(4):
                emit_bd_mm(h)
                emit_bd_write(h)
                emit_read(h)
            emit_ac(0)
            emit_ac(1)
            for h in range(4, 6):
                emit_bd_mm(h)
                emit_bd_write(h)
                emit_read(h)
            emit_merge(0)
            emit_gather(0)
            emit_k(2)
            emit_k(3)
            emit_merge(1)
            emit_gather(1)
            emit_ac(2)
            emit_ac(3)
            for h in range(6, 8):
                emit_bd_mm(h)
                emit_bd_write(h)
                emit_read(h)
            emit_merge(2)
            emit_gather(2)
            emit_v(0)
            emit_v(2)
            emit_ac(4)
            emit_ac(5)
            emit_merge(3)
            emit_gather(3)
            emit_merge(4)
            emit_gather(4)
            emit_v(4)
            emit_v(6)
            emit_ac(6)
            emit_ac(7)
            emit_merge(5)
            emit_gather(5)
            emit_merge(6)
            emit_gather(6)
            emit_merge(7)
            emit_gather(7)
            for hp in range(4):
                emit_pv(hp)

            # ---------------- output projection, bias, store
            for n2 in range(2):
                opsw = mps.tile([128, 1024], dt.float32, tag="m")
                ops = opsw[:, 0:512]
                for c4 in range(4):
                    nc.tensor.matmul(
                        ops,
                        avt[:, c4, 128 * n2:128 * (n2 + 1)],
                        wout[:, c4, :],
                        start=(c4 == 0), stop=(c4 == 3))
                osb = smp.tile([128, 512], dt.float32, tag="osb")
                nc.vector.tensor_add(osb[:], ops, bout[:])
                nc.sync.dma_start(out_d.ap()[128 * n2:128 * (n2 + 1), :],
                                  osb[:])


# ---------------------------------------------------------------- host wrapper
_PROGRAM = None


def _get_program():
    global _PROGRAM
    if _PROGRAM is None:
        _PROGRAM = build_program()
    return _PROGRAM


def make_in_maps(x, memory, W_qkv, W_rel, W_out, b_out, u_emb, v_emb):
    x = np.asarray(x, dtype=F32)
    memory = np.asarray(memory, dtype=F32)
    W_qkv = np.asarray(W_qkv, dtype=F32)
    W_rel = np.asarray(W_rel, dtype=F32)
    W_out = np.asarray(W_out, dtype=F32)
    b_out = np.asarray(b_out, dtype=F32)
    u_emb = np.asarray(u_emb, dtype=F32)
    v_emb = np.asarray(v_emb, dtype=F32)

    R = _positional_encoding()                       # (1024, 22)
    RW = (R @ W_rel).astype(F32)                     # (1024, 512)
    rwt = _chunked(np.ascontiguousarray(RW[::-1].T), 4).astype(BF16)

    wqkv = _chunked(W_qkv, 4).astype(BF16)           # (128, 4, 1536)
    wout = _chunked(W_out, 4).astype(BF16)           # (128, 4, 512)
    bout = np.tile(b_out[None, :], (128, 1)).astype(F32)
    u2 = np.tile(u_emb, 2)[:, None].astype(F32)
    v2 = np.tile(v_emb, 2)[:, None].astype(F32)
    p = np.arange(128)[:, None] % 16
    s = np.arange(16)[None, :]
    gidx = (s * 16 + p).astype(np.int16)             # (128, 16)

    shared = dict(wqkv=wqkv, rwt=rwt, wout=wout, bout=bout, u2=u2, v2=v2,
                  gidx=gidx)
    in_maps = []
    for c in range(B):
        X = np.concatenate([memory[c], x[c]], axis=0)          # (1024, 512)
        xt = _chunked(np.ascontiguousarray(X.T), 4).astype(BF16)
        in_maps.append(dict(xt=xt, **shared))
    return in_maps


def run(in_maps, trace=False, **kw):
    nc = _get_program()
    res = run_bass_kernel_spmd(nc, in_maps, core_ids=list(range(B)),
                               trace=trace, **kw)
    out = np.stack([res.results[c]["out"] for c in range(B)]).astype(F32)
    return out, res


def kernel(x, memory, W_qkv, W_rel, W_out, b_out, u_emb, v_emb):
    in_maps = make_in_maps(x, memory, W_qkv, W_rel, W_out, b_out, u_emb, v_emb)
    out, _ = run(in_maps)
    return out.reshape(B, SEQ, DIM)
